# revision 47
# baseline (speedup 1.0000x reference)
"""Trainium2 Bass kernel for the MemoryModule problem.

Per batch element b (8 of them, one per NeuronCore):
    mk = memory_keys[:, b]  viewed as (Ck=128, M=8192)   [M = T*H*W]
    mv = memory_values[:, b] viewed as (Cv=512, M)
    qk = query_key[b]       viewed as (Ck=128, N=1024)   [N = H*W]
    S  = qk^T @ mk          (N, M)
    P  = softmax(S, axis=-1)
    mem = (P @ mv^T)^T      (Cv, N)
    out[b] = concat([query_value[b], mem], channel axis)

Device dataflow (all transposes done on host / by layout, none on chip):
    - S^T computed directly: S^T tile (128 m, n) = matmul(lhsT=mk_tile, rhs=qk)
      with fp16 inputs (1 cyc/row on the PE vs 4 for fp32).
    - exp on ScalarE (no max subtraction: |S| <~ 70, exp fits fp32/bf16 range),
      written as bf16 P^T tiles.
    - PV: matmul(lhsT=P^T chunk (m,128n), rhs=mv^T tile (m,512c)) in bf16,
      accumulated over the 64 m tiles in PSUM (N=512 = exactly one bank).
      mv^T layout comes straight from a host-side transpose.
    - softmax denominator: VectorE (otherwise idle) accumulates the P^T tiles
      into an SBUF accumulator; at the end of each n-half, 4 tiny PE matmuls
      against a ones vector reduce the 128 partition-partials per n column.
    - normalize with reciprocal + per-partition scalar multiply, DMA out
      as mem^T (N, Cv); host transposes back.

n is processed in halves of 512 so PSUM fits: 3 S^T banks (triple-buffered)
+ 4 PV accumulators + 1 denominator bank = 8.
"""

import os

import numpy as np
import ml_dtypes

T, B, Ck, Cv, H, W = 8, 8, 128, 512, 32, 32
HW = H * W            # 1024  (n dimension)
M = T * HW            # 8192  (memory / contraction dimension)
MT = M // 128         # 64 m-tiles
NQ = 2                # process n in halves
NQS = HW // NQ        # 512 columns of S^T per half
NCH = NQS // 128      # 4 PV accumulators per half
N_CORES = 8

# "f16": fp16 QK matmul (1 cyc/row, ~5e-4 input rounding)
# "f32r": fp32r QK matmul (1 cyc/row, hardware-reduced fp32 precision)
# "f32": exact fp32 QK matmul (4 cyc/row, slow)
QK_MODE = os.environ.get("KERNEL_QK_MODE", "f16")
# >1: repeat the full compute (incl. input DMAs) inside one NEFF via a
# hardware For_i loop, for HW timing via wall-clock deltas. Output is
# identical (rewritten each iteration).
LOOP = int(os.environ.get("KERNEL_LOOP", "1"))
# timing diagnostics: "full" | "dma" (loop only DMAs) | "compute" (DMAs
# hoisted out of the loop, loop only compute)
MODE = os.environ.get("KERNEL_MODE", "full")
# 1: emit an explicit ldweights before each matmul; walrus elides the
# matmul's own (serial) weight load and the standalone one overlaps with
# the previous matmul via the PE background weight buffer (~32 ns/MM).
LDW = os.environ.get("KERNEL_LDW", "1") == "1"
# timing probe: 1 = skip softmax denominator (dn matmuls + reciprocal),
# normalize by a constant instead. WRONG OUTPUT — timing only.
NODN = os.environ.get("KERNEL_NODN", "0") == "1"
# 1: PV matmuls use mv as the stationary operand (static weights, no ACT
# dependency on the weight-load path) and P^T as the moving operand; the
# output comes out as mem^T (Cv, HW) and is normalized on the host using
# the device-computed denominators.
PVB = os.environ.get("KERNEL_PVB", "0") == "1"
# number of DMA descriptors used to load mv (64 = one per m-tile)
MVDMA = int(os.environ.get("KERNEL_MVDMA", "64"))
# 1: store the output as bf16 (host upcasts); halves the tail-DMA bytes
OUTBF = os.environ.get("KERNEL_OUTBF", "1") == "1"
# 1: softmax-denominator accumulator in bf16 (DVE 2x mode, FWL-able dn
# weights); adds <4e-3 denominator error (budget 2e-2)
ACCBF = os.environ.get("KERNEL_ACCBF", "0") == "1"

_CACHE = {}
LAST_RESULTS = None


def _build_nc(
    qk_mode,
    loop=1,
    mode="full",
    ldw=LDW,
    nodn=None,
    interleave=None,
    pvb=None,
    dnlate=None,
    explead=None,
    mvdma=None,
    outbf=None,
    accbf=None,
):
    if mvdma is None:
        mvdma = MVDMA
    if outbf is None:
        outbf = OUTBF
    if accbf is None:
        accbf = ACCBF
    if nodn is None:
        nodn = NODN
    if interleave is None:
        interleave = os.environ.get("KERNEL_INTERLEAVE_Q", "1") == "1"
    if pvb is None:
        pvb = PVB
    if dnlate is None:
        dnlate = os.environ.get("KERNEL_DNLATE", "0") == "1"
    if explead is None:
        explead = int(os.environ.get("KERNEL_EXPLEAD", "2"))
    assert not (dnlate and not pvb), "dnlate requires pvb (host normalize)"
    import concourse.tile as tile
    import concourse.mybir as mybir
    from concourse import bacc

    f32 = mybir.dt.float32
    bf16 = mybir.dt.bfloat16
    f16 = mybir.dt.float16
    qk_dt = {"f16": f16, "f32r": f32, "f32": f32}[qk_mode]

    nc = bacc.Bacc()

    def mm(out_ap, w_ap, x_ap, **kw):
        if ldw and w_ap.dtype not in (mybir.dt.float32, mybir.dt.float32r):
            nc.tensor.ldweights(w_ap)
        nc.tensor.matmul(out_ap, w_ap, x_ap, **kw)

    qk_d = nc.dram_tensor("qk", [Ck, HW], qk_dt, kind="ExternalInput")
    mk_d = nc.dram_tensor("mk", [Ck, M], qk_dt, kind="ExternalInput")
    mv_d = nc.dram_tensor("mv", [M, Cv], bf16, kind="ExternalInput")
    out_dt = bf16 if outbf else f32
    if pvb:
        out_d = nc.dram_tensor("out", [Cv, HW], out_dt, kind="ExternalOutput")
        dn_d = nc.dram_tensor("dn", [128, NQ * NCH], f32, kind="ExternalOutput")
    else:
        out_d = nc.dram_tensor("out", [HW, Cv], out_dt, kind="ExternalOutput")

    mv_tiled = mv_d.rearrange("(mt p) c -> mt p c", p=128)  # (64, 128, 512)
    mv_chunked = mv_d.rearrange("(mt p) c -> p mt c", p=128)  # (128, 64, 512)

    Exp = mybir.ActivationFunctionType.Exp
    AluOp = mybir.AluOpType

    def emit_dma(nc, tc, big):
        qk_sb = big.tile([Ck, HW], qk_dt, tag="qk_sb", name="qk_sb")
        nc.sync.dma_start(qk_sb[:], qk_d[:])
        mk_sb = big.tile([Ck, M], qk_dt, tag="mk_sb", name="mk_sb")
        # split so the first S^T matmuls don't wait for the whole tensor
        for i in range(8):
            nc.sync.dma_start(
                mk_sb[:, i * HW : (i + 1) * HW], mk_d[:, i * HW : (i + 1) * HW]
            )
        mv_sb = big.tile([128, MT, Cv], bf16, tag="mv_sb", name="mv_sb")
        if mvdma == MT:
            for m in range(MT):
                nc.sync.dma_start(mv_sb[:, m], mv_tiled[m])
        else:
            step = MT // mvdma
            for i in range(mvdma):
                nc.sync.dma_start(
                    mv_sb[:, i * step : (i + 1) * step],
                    mv_chunked[:, i * step : (i + 1) * step],
                )
        return qk_sb, mk_sb, mv_sb

    def body(nc, tc, big, ptp, accp, outp, smallp, stp, pvp, dnp, tiles):
        qk_sb, mk_sb, mv_sb = tiles
        ones_sb = big.tile(
            [128, 1], bf16 if accbf else f32, tag="ones_sb", name="ones_sb"
        )
        nc.vector.memset(ones_sb[:], 1.0)

        def mm_cast(ap):
            if qk_mode == "f32r":
                return ap.bitcast(mybir.dt.float32r)
            return ap

        class QState:
            def __init__(self, q):
                self.q = q
                self.sts = {}
                self.pts = {}

            def emit_st(self, m):
                st = stp.tile(
                    [128, NQS], f32, tag="st", name=f"st_q{self.q}_m{m}"
                )
                mm(
                    st[:],
                    mm_cast(mk_sb[:, m * 128 : (m + 1) * 128]),
                    mm_cast(qk_sb[:, self.q * NQS : (self.q + 1) * NQS]),
                    start=True,
                    stop=True,
                )
                self.sts[m] = st

            def emit_exp(self, m):
                pt = ptp.tile(
                    [128, NQS], bf16, tag="pt", name=f"pt_q{self.q}_m{m}"
                )
                nc.scalar.activation(pt[:], self.sts.pop(m)[:], Exp)
                self.pts[m] = pt

            def emit_acc(self, m):
                # VectorE: accumulate exp tiles for the softmax denominator
                if m == 0:
                    nc.vector.tensor_copy(self.acc[:], self.pts[m][:])
                else:
                    nc.vector.tensor_tensor(
                        self.acc[:], self.acc[:], self.pts[m][:], AluOp.add
                    )

            def prologue(self):
                q = self.q
                self.pv = [
                    pvp.tile([128, NQS], f32, tag=f"pv{i}", name=f"pv_q{q}_{i}")
                    for i in range(NCH)
                ]
                self.acc = accp.tile(
                    [128, NQS], bf16 if accbf else f32, tag="acc", name=f"acc_q{q}"
                )
                # software pipeline: PE always has the next S^T ready, ACT
                # runs `explead` tiles ahead of the PV consumers so PV's
                # LDWEIGHTS never waits on an unsatisfied semaphore
                self.emit_st(0)
                self.emit_st(1)
                self.emit_exp(0)
                self.emit_st(2)
                self.emit_exp(1)
                self.emit_st(3)
                if explead >= 3:
                    self.emit_exp(2)

            def mloop(self, inject=None):
                for m in range(MT):
                    if m == 3 and inject is not None:
                        inject()
                    ptm = self.pts[m]
                    for nch in range(NCH):
                        if pvb:
                            mm(
                                self.pv[nch][:],
                                mv_sb[:, m, nch * 128 : (nch + 1) * 128],
                                ptm[:],
                                start=(m == 0),
                                stop=(m == MT - 1),
                            )
                        else:
                            mm(
                                self.pv[nch][:],
                                ptm[:, nch * 128 : (nch + 1) * 128],
                                mv_sb[:, m],
                                start=(m == 0),
                                stop=(m == MT - 1),
                            )
                    if m + explead < MT:
                        self.emit_exp(m + explead)
                    self.emit_acc(m)
                    del self.pts[m]
                    if m + 4 < MT:
                        self.emit_st(m + 4)

            def epilogue_dn(self):
                q = self.q
                if nodn:
                    return
                dn = dnp.tile([128, NCH], f32, tag="dn", name=f"dn_q{q}")
                for nch in range(NCH):
                    mm(
                        dn[:, nch : nch + 1],
                        self.acc[:, nch * 128 : (nch + 1) * 128],
                        ones_sb[:],
                        start=True,
                        stop=True,
                    )
                dns = smallp.tile(
                    [128, NCH], f32, tag="dns", name=f"dns_q{q}"
                )
                nc.vector.tensor_copy(dns[:], dn[:])
                nc.sync.dma_start(dn_d[:, q * NCH : (q + 1) * NCH], dns[:])

            def epilogue_out(self):
                q = self.q
                # PSUM evacuation as plain copies split across DVE and ACT;
                # denominators go out raw, the host divides.
                for nch in range(NCH):
                    o = outp.tile([128, NQS], out_dt, tag="o", name=f"o_q{q}_{nch}")
                    if nch % 2 == 0:
                        nc.vector.tensor_copy(o[:], self.pv[nch][:])
                    else:
                        nc.scalar.activation(
                            o[:],
                            self.pv[nch][:],
                            mybir.ActivationFunctionType.Copy,
                        )
                    nc.sync.dma_start(
                        out_d[nch * 128 : (nch + 1) * 128, q * NQS : (q + 1) * NQS],
                        o[:],
                    )

            def epilogue(self):
                q = self.q
                if pvb:
                    if not dnlate:
                        self.epilogue_dn()
                    self.epilogue_out()
                    return
                if nodn:
                    recip = smallp.tile(
                        [128, NCH], f32, tag="recip", name=f"recip_q{q}"
                    )
                    nc.vector.memset(recip[:], 1.0 / 8192.0)
                else:
                    # denominator: reduce acc over partitions with tiny matmuls
                    dn = dnp.tile([128, NCH], f32, tag="dn", name=f"dn_q{q}")
                    for nch in range(NCH):
                        mm(
                            dn[:, nch : nch + 1],
                            self.acc[:, nch * 128 : (nch + 1) * 128],
                            ones_sb[:],
                            start=True,
                            stop=True,
                        )
                    recip = smallp.tile(
                        [128, NCH], f32, tag="recip", name=f"recip_q{q}"
                    )
                    nc.vector.reciprocal(recip[:], dn[:])
                for nch in range(NCH):
                    o = outp.tile([128, Cv], out_dt, tag="o", name=f"o_q{q}_{nch}")
                    # split the tail normalize across DVE and ACT so the
                    # final PSUM evacuation halves in wall-clock
                    if nch % 2 == 0:
                        nc.vector.tensor_scalar_mul(
                            o[:], self.pv[nch][:], recip[:, nch : nch + 1]
                        )
                    else:
                        nc.scalar.activation(
                            o[:],
                            self.pv[nch][:],
                            mybir.ActivationFunctionType.Copy,
                            scale=recip[:, nch : nch + 1],
                        )
                    n0 = q * NQS + nch * 128
                    nc.sync.dma_start(out_d[n0 : n0 + 128, :], o[:])

        # INTERLEAVE_Q: emit the next half's S^T prologue before this half's
        # epilogue so PE isn't FIFO-blocked behind the denominator matmuls
        # (which wait on the DVE accumulation tail)
        states = [QState(q) for q in range(NQ)]
        states[0].prologue()
        pending_dn = None
        for q in range(NQ):
            states[q].mloop(inject=pending_dn)
            pending_dn = None
            if interleave and q + 1 < NQ:
                states[q + 1].prologue()
            states[q].epilogue()
            if pvb and dnlate:
                if q + 1 < NQ:
                    pending_dn = states[q].epilogue_dn
                else:
                    states[q].epilogue_dn()
            if not interleave and q + 1 < NQ:
                states[q + 1].prologue()

    with tile.TileContext(nc) as tc:
        with (
            tc.tile_pool(name="big", bufs=1) as big,
            tc.tile_pool(name="ptp", bufs=6) as ptp,
            tc.tile_pool(name="accp", bufs=3) as accp,
            tc.tile_pool(name="outp", bufs=6) as outp,
            tc.tile_pool(name="smallp", bufs=4) as smallp,
            tc.tile_pool(name="stp", bufs=3, space="PSUM") as stp,
            tc.tile_pool(name="pvp", bufs=1, space="PSUM") as pvp,
            tc.tile_pool(name="dnp", bufs=1, space="PSUM") as dnp,
        ):
            if mode in ("mmonly", "mmnodma", "mmpv") and loop > 1:
                # pure PE stream: same LDW+MM pair count/shapes as the real
                # kernel, but no ACT/DVE in the loop (weights from a fixed
                # dummy tile). mmnodma hoists input DMAs out of the loop;
                # mmpv additionally drops the S^T matmuls.
                include_st = mode != "mmpv"
                tiles = emit_dma(nc, tc, big)
                qk_sb, mk_sb, mv_sb = tiles
                dummy_pt = big.tile([128, NQS], bf16, tag="dummy_pt", name="dummy_pt")
                nc.vector.memset(dummy_pt[:], 0.001)
                with tc.For_i(0, loop, 1):
                    for q in range(NQ):
                        pv = [
                            pvp.tile([128, NQS], f32, tag=f"pv{i}", name=f"mm_pv_q{q}_{i}")
                            for i in range(NCH)
                        ]
                        for m in range(MT):
                            if include_st:
                                st = stp.tile([128, NQS], f32, tag="st", name=f"mm_st_q{q}_m{m}")
                                mm(
                                    st[:],
                                    mk_sb[:, m * 128 : (m + 1) * 128],
                                    qk_sb[:, q * NQS : (q + 1) * NQS],
                                    start=True,
                                    stop=True,
                                )
                            for nch in range(NCH):
                                mm(
                                    pv[nch][:],
                                    dummy_pt[:, nch * 128 : (nch + 1) * 128],
                                    mv_sb[:, m],
                                    start=(m == 0),
                                    stop=(m == MT - 1),
                                )
                        for nch in range(NCH):
                            o = outp.tile([128, Cv], f32, tag="o", name=f"mm_o_q{q}_{nch}")
                            nc.vector.tensor_copy(o[:], pv[nch][:])
                            nc.sync.dma_start(
                                out_d[(q * NCH + nch) * 128 : (q * NCH + nch + 1) * 128, :],
                                o[:],
                            )
            elif mode == "compute" and loop > 1:
                tiles = emit_dma(nc, tc, big)
                with tc.For_i(0, loop, 1):
                    body(nc, tc, big, ptp, accp, outp, smallp, stp, pvp, dnp, tiles)
            elif mode == "dma" and loop > 1:
                with tc.For_i(0, loop, 1):
                    emit_dma(nc, tc, big)
                    # include the output-store traffic too
                    for j in range(8):
                        o = outp.tile([128, Cv], f32, tag="o", name=f"o_{j}")
                        nc.vector.memset(o[:], float(j))
                        nc.sync.dma_start(out_d[j * 128 : (j + 1) * 128, :], o[:])
            else:
                loop_ctx = tc.For_i(0, loop, 1) if loop > 1 else None
                with (loop_ctx if loop_ctx is not None else _null()):
                    tiles = emit_dma(nc, tc, big)
                    body(nc, tc, big, ptp, accp, outp, smallp, stp, pvp, dnp, tiles)

    nc.finalize()
    return nc


class _null:
    def __enter__(self):
        return None

    def __exit__(self, *a):
        return False


def _get_nc():
    key = (
        "nc", QK_MODE, LOOP, MODE, LDW, NODN, PVB, MVDMA, OUTBF, ACCBF,
        os.environ.get("KERNEL_DNLATE", "0"),
        os.environ.get("KERNEL_EXPLEAD", "2"),
        os.environ.get("KERNEL_INTERLEAVE_Q", "1"),
    )
    if key not in _CACHE:
        _CACHE[key] = _build_nc(QK_MODE, LOOP, MODE)
    return _CACHE[key]


def _prep_core_inputs(memory_keys, memory_values, query_key, b):
    np_qk_dt = np.float16 if QK_MODE == "f16" else np.float32
    # astype on the transposed view fuses cast+copy in one pass
    qk = query_key[b].reshape(Ck, HW).astype(np_qk_dt)
    mk = memory_keys[:, b].transpose(1, 0, 2, 3).astype(np_qk_dt).reshape(Ck, M)
    mv = (
        memory_values[:, b]
        .transpose(0, 2, 3, 1)
        .astype(ml_dtypes.bfloat16)
        .reshape(M, Cv)
    )
    return {"qk": qk, "mk": mk, "mv": mv}


_RUNNER = {}


def _get_runner():
    """Build the sharded PJRT callable once and reuse it — the generic
    run_bass_kernel_spmd path re-traces jax.jit on every call (~2 s)."""
    if "r" not in _RUNNER:
        import jax
        from jax.sharding import Mesh, PartitionSpec, NamedSharding
        from jax.experimental.shard_map import shard_map

        import concourse.mybir as mybir
        from concourse import bass2jax
        from concourse.bass2jax import _bass_exec_p, install_neuronx_cc_hook

        nc = _get_nc()
        install_neuronx_cc_hook()
        pname = nc.partition_id_tensor.name if nc.partition_id_tensor else None
        in_names, out_names, out_avals = [], [], []
        for alloc in nc.m.functions[0].allocations:
            if not isinstance(alloc, mybir.MemoryLocationSet):
                continue
            name = alloc.memorylocations[0].name
            if alloc.kind == "ExternalInput":
                if name != pname:
                    in_names.append(name)
            elif alloc.kind == "ExternalOutput":
                out_names.append(name)
                out_avals.append(
                    jax.core.ShapedArray(
                        tuple(alloc.tensor_shape), mybir.dt.np(alloc.dtype)
                    )
                )
        n_params = len(in_names)
        all_in = list(in_names) + list(out_names) + ([pname] if pname else [])

        def _body(*args):
            operands = list(args)
            if pname is not None:
                operands.append(bass2jax.partition_id_tensor())
            return tuple(
                _bass_exec_p.bind(
                    *operands,
                    out_avals=tuple(out_avals),
                    in_names=tuple(all_in),
                    out_names=tuple(out_names),
                    lowering_input_output_aliases=(),
                    sim_require_finite=True,
                    sim_require_nnan=True,
                    nc=nc,
                )
            )

        mesh = Mesh(np.asarray(jax.devices()[:N_CORES]), ("core",))
        n_outs = len(out_names)
        sharded = jax.jit(
            shard_map(
                _body,
                mesh=mesh,
                in_specs=(PartitionSpec("core"),) * (n_params + n_outs),
                out_specs=(PartitionSpec("core"),) * n_outs,
                check_rep=False,
            ),
            keep_unused=True,
        )
        sh = NamedSharding(mesh, PartitionSpec("core"))
        zeros = [
            jax.device_put(
                np.zeros((N_CORES * a.shape[0], *a.shape[1:]), a.dtype), sh
            )
            for a in out_avals
        ]
        _RUNNER["r"] = (sharded, sh, in_names, zeros)
    return _RUNNER["r"]


def kernel(memory_keys, memory_values, query_key, query_value):
    global LAST_RESULTS
    memory_keys = np.asarray(memory_keys, dtype=np.float32)
    memory_values = np.asarray(memory_values, dtype=np.float32)
    query_key = np.asarray(query_key, dtype=np.float32)
    query_value = np.asarray(query_value, dtype=np.float32)

    in_maps = [
        _prep_core_inputs(memory_keys, memory_values, query_key, b)
        for b in range(N_CORES)
    ]
    try:
        import jax

        sharded, sh, in_names, zeros = _get_runner()
        dev_in = [
            jax.device_put(
                np.concatenate([in_maps[c][n] for c in range(N_CORES)], 0), sh
            )
            for n in in_names
        ]
        outs = sharded(*dev_in, *zeros)
        if PVB:
            out_full = np.asarray(outs[0]).astype(np.float32).reshape(N_CORES, Cv, HW)
            dn_full = np.asarray(outs[1]).reshape(N_CORES, 128, NQ * NCH)
        else:
            out_full = (
                np.asarray(outs[0]).astype(np.float32).reshape(N_CORES, HW, Cv)
            )
    except Exception:
        # conservative fallback: the generic (slower per call) exec path
        from concourse.bass_utils import run_bass_kernel_spmd

        res = run_bass_kernel_spmd(
            _get_nc(), in_maps, core_ids=list(range(N_CORES))
        )
        LAST_RESULTS = res
        out_full = np.stack([res.results[b]["out"] for b in range(N_CORES)])
        if PVB:
            dn_full = np.stack([res.results[b]["dn"] for b in range(N_CORES)])

    if PVB:
        denom = (
            dn_full.reshape(N_CORES, 128, NQ, NCH)
            .transpose(0, 2, 3, 1)
            .reshape(N_CORES, HW)
        )
        mem = (out_full / denom[:, None, :]).reshape(N_CORES, Cv, H, W)
        mem = mem.astype(np.float32)
    else:
        mem = np.stack(
            [out_full[b].T.reshape(Cv, H, W) for b in range(N_CORES)]
        ).astype(np.float32)
    return np.concatenate([query_value, mem], axis=1)

